# revision 6
# baseline (speedup 1.0000x reference)
"""Trainium2 Bass kernel for nn_Decoder (2-layer LSTM decoder with BatchNorm +
LockedDropout + vocab projection), tensor-parallel over the hidden dim across
8 NeuronCores.

Contract: kernel(**inputs) takes FULL inputs (as produced by setup_inputs())
and returns the FULL [B*T, V] float32 output.

Sharding:
  - Each core owns a 128-unit slice of the hidden dim for BOTH LSTM layers
    (gates i,f,g,o for those units) -> gate matmuls have M=128 per gate with
    full batch B=256 as the moving dim (full PE width, BN stats exact over
    the full batch).
  - ONE AllGather per step (ncfw, through HBM) carries four [P, B] slices:
    h1(t), y1(t) staged by this step's cell1, and h2(t-1), y2(t-1) staged at
    the end of the previous step.  h2(t-1) is not needed until the layer-2
    gates of step t, and y2(t-1) not until the projection of step t-1 two
    steps later, so folding them into the next step's collective halves the
    number of per-step barriers without lengthening the recurrence.
  - The vocab projection is sharded over V (1250 per core).

Schedule (per step t) -- emission order is the engines' execution order:
  cell1(t) -> stage(h1;y1) -> AG(t) -> readbacks (y1g+h1g on the SP ring,
  h2g+y2g on the DVE ring) -> [PE filler while AG flies: proj(t-2) chunks
  0-1, ih1(t+1) pre-emit, proj chunks 2-3] -> L2 gates (gate-major:
  ih2+hh2 per gate so ACT starts cell2 after 16 matmuls) -> cell2(t) ->
  stage(h2;y2 for AG(t+1)) -> hh1(t+1) -> proj chunks 4-5 (cover the
  cell1(t+1) ACT/DVE tail).

  The PE stream is in-order: everything emitted between AG(t) and the L2
  gates is AG-independent work (projection of step t-2 whose y2g landed a
  full step ago, and the x-side of next step's layer-1 gates), sized to
  cover the collective's ~7us latency so the PE never idles and stays at
  its top p-state (the cost model drops the PE clock to half for 3us after
  any idle gap).

PSUM: 6 banks rotate over the LSTM gate groups (layer1(t), layer2(t),
layer1(t+1) pre-emitted), one accumulation group per bank (start on the
bank's first matmul, gates 0/1 only); 2 banks double-buffer the projection.

Matmul/transport dtype is bf16: full PE rate, half the collective/DMA
bytes, measured rel err ~1.2e-2 against the fp32 reference (gate 2e-2).
"""

import contextlib
import os
import sys

sys.path.insert(0, "/opt/trn_rl_repo")

import ml_dtypes
import numpy as np

import concourse.bass as bass
import concourse.tile as tile
from concourse import bacc, mybir
from concourse.bass_utils import run_bass_kernel_spmd

F32 = mybir.dt.float32
F32R = mybir.dt.float32r
BF16 = mybir.dt.bfloat16

DT_MM_NAME = os.environ.get("TRN_DT_MM", "bf16")
DT_MM = {"f32r": F32R, "bf16": BF16, "f32": F32}[DT_MM_NAME]

B, L, E, H, V = 256, 20, 512, 1024, 10000
T = int(os.environ.get("TRN_T", L + 1))
NCORE = 8
P = 128
HS = H // NCORE          # 128 hidden units per core per layer
VS = V // NCORE          # 1250 vocab slots per core
NKE = E // P             # 4 k-tiles over E
NKH = H // P             # 8 k-tiles over H
BN_EPS = 1e-5
# projection N-chunks of VS=1250 (each >=256 so fp32r runs full rate)
NCHUNKS = [(0, 418), (418, 416), (834, 416)]

LAST_EXEC_NS = None
# TRN_FAKE_AG=1 replaces collectives with a local DMA (debug-only)
FAKE_AG = os.environ.get("TRN_FAKE_AG", "0") == "1"

_CACHE = {}


def _fp32r_round(x):
    hi = x.astype(ml_dtypes.bfloat16).astype(np.float32)
    lo = (x - hi).astype(ml_dtypes.bfloat16).astype(np.float32)
    return hi + lo


def build_bass():
    nc = bacc.Bacc("TRN2", target_bir_lowering=False, num_devices=NCORE)
    dt = DT_MM
    ddt = dt

    # ---------------- DRAM I/O ----------------
    d_xT = nc.dram_tensor("xT", [T, NKE, P, B], ddt, kind="ExternalInput")
    d_wih1 = nc.dram_tensor("wih1", [4, NKE, P, HS], ddt, kind="ExternalInput")
    d_whh1 = nc.dram_tensor("whh1", [4, NKH, P, HS], ddt, kind="ExternalInput")
    d_wih2 = nc.dram_tensor("wih2", [4, NKH, P, HS], ddt, kind="ExternalInput")
    d_whh2 = nc.dram_tensor("whh2", [4, NKH, P, HS], ddt, kind="ExternalInput")
    d_woutT = nc.dram_tensor("woutT", [NKH, P, VS], ddt, kind="ExternalInput")
    d_bias1 = nc.dram_tensor("bias1", [HS, 4], F32, kind="ExternalInput")
    d_bias2 = nc.dram_tensor("bias2", [HS, 4], F32, kind="ExternalInput")
    d_gb1 = nc.dram_tensor("gb1", [HS, 2], F32, kind="ExternalInput")
    d_gb2 = nc.dram_tensor("gb2", [HS, 2], F32, kind="ExternalInput")
    d_m1T = nc.dram_tensor("m1T", [HS, B], F32, kind="ExternalInput")
    d_m2T = nc.dram_tensor("m2T", [HS, B], F32, kind="ExternalInput")
    d_out = nc.dram_tensor("out", [B * T, VS], F32, kind="ExternalOutput")

    # collective bounce buffers (inputs must be Local, outputs Shared)
    # slot layout: 0 = h1(t), 1 = y1(t), 2 = h2(t-1), 3 = y2(t-1)
    RING = 3
    agi = [nc.dram_tensor(f"agi{j}", [4, P, B], dt, kind="Internal")
           for j in range(RING)]
    ago = [nc.dram_tensor(f"ago{j}", [NCORE, 4, P, B], dt,
                          kind="Internal", addr_space="Shared")
           for j in range(RING)]

    # only SP and ACT have HWDGE rings (gpsimd's SWDGE is reserved for the
    # collectives): SP carries the latency-critical stage + y1g/h1g chain,
    # ACT carries weights, h2g/y2g readbacks, x loads and proj stores.
    dma_sp = nc.sync.dma_start
    dma_act = nc.scalar.dma_start
    dma_dve = dma_act
    dma_pe = dma_act
    dma_pl = dma_act

    with tile.TileContext(nc) as tc:
        with contextlib.ExitStack() as ctx:
            smalls = ctx.enter_context(tc.tile_pool(name="smalls", bufs=1))
            wts = ctx.enter_context(tc.tile_pool(name="wts", bufs=1))
            xpool = ctx.enter_context(tc.tile_pool(name="xpool", bufs=3))
            gp_h1 = ctx.enter_context(tc.tile_pool(name="g_h1", bufs=2))
            gp_y1 = ctx.enter_context(tc.tile_pool(name="g_y1", bufs=2))
            gp_h2 = ctx.enter_context(tc.tile_pool(name="g_h2", bufs=2))
            gp_y2 = ctx.enter_context(tc.tile_pool(name="g_y2", bufs=2))
            cell = ctx.enter_context(tc.tile_pool(name="cell", bufs=2))
            slpool = ctx.enter_context(tc.tile_pool(name="slp", bufs=2))
            state = ctx.enter_context(tc.tile_pool(name="state", bufs=1))
            psumg = ctx.enter_context(
                tc.tile_pool(name="psumg", bufs=6, space="PSUM"))
            psumP = ctx.enter_context(
                tc.tile_pool(name="psumP", bufs=2, space="PSUM"))
            outp = ctx.enter_context(tc.tile_pool(name="outp", bufs=3))

            # resident weights
            w_ih1 = wts.tile([P, 4, NKE, HS], dt)
            w_hh1 = wts.tile([P, 4, NKH, HS], dt)
            w_ih2 = wts.tile([P, 4, NKH, HS], dt)
            w_hh2 = wts.tile([P, 4, NKH, HS], dt)
            w_out = wts.tile([P, NKH, VS], dt)

            def load_weight(dstq, dst, dram):
                dstq(dst[:], dram[:].rearrange("g k p m -> p g k m"))

            def load_x(t):
                x_t = xpool.tile([P, NKE, B], dt, tag="x", name=f"x_{t}")
                dma_pl(x_t[:], d_xT[t][:].rearrange("k p b -> p k b"))
                return x_t

            # prologue: wih1 + x0 unblock the PE first (split across both
            # rings), then the remaining weights stream on ACT in first-use
            # order while the smalls ride SP.
            load_weight(dma_sp, w_ih1, d_wih1)
            x_t = load_x(0)
            load_weight(dma_act, w_ih2, d_wih2)
            load_weight(dma_act, w_hh1, d_whh1)
            load_weight(dma_act, w_hh2, d_whh2)
            dma_act(w_out[:], d_woutT[:].rearrange("k p v -> p k v"))

            b1 = smalls.tile([HS, 4], F32)
            b2 = smalls.tile([HS, 4], F32)
            gb1 = smalls.tile([HS, 2], F32)
            gb2 = smalls.tile([HS, 2], F32)
            m1 = smalls.tile([HS, B], F32)
            m2 = smalls.tile([HS, B], F32)
            epst = smalls.tile([P, 1], F32)
            for dst, src in ((b1, d_bias1), (b2, d_bias2), (gb1, d_gb1),
                             (gb2, d_gb2), (m1, d_m1T), (m2, d_m2T)):
                dma_sp(dst[:], src[:])
            nc.vector.memset(epst[:], BN_EPS)

            # zero-fill AG(0)'s h2/y2 slots (transported but never read back)
            zt = smalls.tile([P, 2, B], dt)
            nc.vector.memset(zt[:], 0.0)
            agi0r = agi[0][:].rearrange("s p b -> p s b")
            dma_sp(agi0r[:, 2:4], zt[:])

            # persistent state
            c1 = state.tile([P, B], F32)
            c2 = state.tile([P, B], F32)
            nc.vector.memset(c1[:], 0.0)
            nc.vector.memset(c2[:], 0.0)

            def lstm_cell(bias, gbv, mask, c_st, pgs, hy_out, t):
                """One LSTM cell + BatchNorm + dropout-mask.

                psum packing: pgA=(i,g), pgB=(f,o); gate order i=0 f=1 g=2 o=3.
                pgs: (pgA, pgB) whose accumulation is complete (stop emitted).
                hy_out: [P, 2, B] dt tile; slice 0 <- h (cast), slice 1 <- y.
                """
                pgA, pgB = pgs
                i_t = cell.tile([P, B], F32, tag="i", name=f"i_{t}")
                f_t = cell.tile([P, B], F32, tag="f", name=f"f_{t}")
                g_t = cell.tile([P, B], F32, tag="g", name=f"g_{t}")
                o_t = cell.tile([P, B], F32, tag="o", name=f"o_{t}")
                Sig = mybir.ActivationFunctionType.Sigmoid
                Tanh = mybir.ActivationFunctionType.Tanh
                nc.scalar.activation(i_t[:], pgA[:, 0], Sig, bias=bias[:, 0:1])
                nc.scalar.activation(g_t[:], pgA[:, 1], Tanh, bias=bias[:, 2:3])
                nc.scalar.activation(f_t[:], pgB[:, 0], Sig, bias=bias[:, 1:2])
                nc.scalar.activation(o_t[:], pgB[:, 1], Sig, bias=bias[:, 3:4])

                ig = cell.tile([P, B], F32, tag="ig", name=f"ig_{t}")
                nc.vector.tensor_mul(ig[:], i_t[:], g_t[:])
                fc = cell.tile([P, B], F32, tag="fc", name=f"fc_{t}")
                nc.vector.tensor_mul(fc[:], f_t[:], c_st[:])
                nc.vector.tensor_add(c_st[:], ig[:], fc[:])
                tnc = cell.tile([P, B], F32, tag="tc", name=f"tc_{t}")
                nc.scalar.activation(tnc[:], c_st[:], Tanh)
                h_f = cell.tile([P, B], F32, tag="h", name=f"h_{t}")
                nc.vector.tensor_mul(h_f[:], o_t[:], tnc[:])
                nc.scalar.activation(hy_out[:, 0], h_f[:],
                                     mybir.ActivationFunctionType.Identity)
                # BN stats over batch (free dim)
                st6 = cell.tile([P, 6], F32, tag="st", name=f"st_{t}")
                nc.vector.bn_stats(st6[:], h_f[:])
                mv = cell.tile([P, 2], F32, tag="mv", name=f"mv_{t}")
                nc.vector.bn_aggr(mv[:], st6[:])
                # rstd = rsqrt(var + eps), DVE-only (fast-inverse-sqrt +
                # 2 Newton steps) -- keeps the ACT LUT on sigmoid/tanh, no
                # table swaps.
                I32 = mybir.dt.int32
                v_t = cell.tile([P, 1], F32, tag="vv", name=f"vv_{t}")
                nc.vector.tensor_scalar_add(v_t[:], mv[:, 1:2], BN_EPS)
                r_a = cell.tile([P, 1], F32, tag="ra", name=f"ra_{t}")
                r_b = cell.tile([P, 1], F32, tag="rb", name=f"rb_{t}")
                ui = cell.tile([P, 1], I32, tag="ui", name=f"ui_{t}")
                nc.vector.tensor_scalar(ui[:], v_t[:].bitcast(I32), 1, None,
                                        op0=mybir.AluOpType.logical_shift_right)
                nc.vector.tensor_scalar(r_a[:].bitcast(I32), ui[:],
                                        -1, 0x5F3759DF,
                                        op0=mybir.AluOpType.mult,
                                        op1=mybir.AluOpType.add)
                rr = cell.tile([P, 1], F32, tag="rr", name=f"rr_{t}")
                ww = cell.tile([P, 1], F32, tag="ww", name=f"ww_{t}")
                r_cur, r_nxt = r_a, r_b
                for it in range(2):
                    nc.vector.tensor_mul(rr[:], r_cur[:], r_cur[:])
                    nc.vector.scalar_tensor_tensor(
                        ww[:], rr[:], -0.5, v_t[:],
                        op0=mybir.AluOpType.mult, op1=mybir.AluOpType.mult)
                    nc.vector.scalar_tensor_tensor(
                        r_nxt[:], ww[:], 1.5, r_cur[:],
                        op0=mybir.AluOpType.add, op1=mybir.AluOpType.mult)
                    r_cur, r_nxt = r_nxt, r_cur
                a_v = cell.tile([P, 1], F32, tag="av", name=f"av_{t}")
                nc.vector.tensor_mul(a_v[:], r_cur[:], gbv[:, 0:1])
                ma = cell.tile([P, 1], F32, tag="ma", name=f"ma_{t}")
                nc.vector.tensor_mul(ma[:], mv[:, 0:1], a_v[:])
                b_v = cell.tile([P, 1], F32, tag="bv", name=f"bv_{t}")
                nc.vector.tensor_sub(b_v[:], gbv[:, 1:2], ma[:])
                yt = cell.tile([P, B], F32, tag="yt", name=f"yt_{t}")
                nc.scalar.activation(yt[:], h_f[:],
                                     mybir.ActivationFunctionType.Identity,
                                     bias=b_v[:], scale=a_v[:])
                nc.vector.tensor_mul(hy_out[:, 1], yt[:], mask[:])

            def emit_ih1(t, x_t, stop):
                # x-side of layer-1 gates for step t.  One accumulation group
                # per PSUM bank: start on the bank's first matmul (gates 0/1),
                # stop on each gate's last matmul (deferred to the hh matmuls
                # unless `stop`).
                pgA = psumg.tile([P, 2, B], F32, tag="pg", name=f"pgA_1_{t}")
                pgB = psumg.tile([P, 2, B], F32, tag="pg", name=f"pgB_1_{t}")
                gloc = {0: (pgA, 0), 2: (pgA, 1), 1: (pgB, 0), 3: (pgB, 1)}
                for gate in (0, 2, 1, 3):
                    tl, sub = gloc[gate]
                    for k in range(NKE):
                        nc.tensor.matmul(
                            tl[:, sub], w_ih1[:, gate, k], x_t[:, k],
                            start=(k == 0 and gate in (0, 1)),
                            stop=(stop and k == NKE - 1 and gate in (2, 3)))
                return pgA, pgB

            def emit_hh1(pgs, h1g):
                # finish layer-1 gate accumulation with the recurrent part;
                # gate-major so ACT can start cell1 after each gate's 8th mm
                pgA, pgB = pgs
                gloc = {0: (pgA, 0), 2: (pgA, 1), 1: (pgB, 0), 3: (pgB, 1)}
                for gate in (0, 2, 1, 3):
                    tl, sub = gloc[gate]
                    for k in range(NKH):
                        nc.tensor.matmul(
                            tl[:, sub], w_hh1[:, gate, k], h1g[:, k],
                            start=False,
                            stop=(k == NKH - 1))

            def emit_l2(t, y1g, h2g):
                # layer-2 gates, gate-major: each gate's full accumulation
                # (ih2 over y1g, then hh2 over h2g) completes in 16 matmuls so
                # cell2's ACT ops start after ~1.7us instead of ~6.8us.
                # One accumulation group per bank: start on the bank's first
                # matmul (gates 0/1), stop on each gate's last.
                pgA = psumg.tile([P, 2, B], F32, tag="pg", name=f"pgA_2_{t}")
                pgB = psumg.tile([P, 2, B], F32, tag="pg", name=f"pgB_2_{t}")
                gloc = {0: (pgA, 0), 2: (pgA, 1), 1: (pgB, 0), 3: (pgB, 1)}
                last = NKH - 1
                for gate in (0, 2, 1, 3):
                    tl, sub = gloc[gate]
                    for k in range(NKH):
                        nc.tensor.matmul(
                            tl[:, sub], w_ih2[:, gate, k], y1g[:, k],
                            start=(k == 0 and gate in (0, 1)),
                            stop=(h2g is None and k == last))
                    if h2g is not None:
                        for k in range(NKH):
                            nc.tensor.matmul(
                                tl[:, sub], w_hh2[:, gate, k], h2g[:, k],
                                start=False, stop=(k == last))
                return pgA, pgB

            # out rows at step tp as [p, bh, v] (partition-major SBUF view)
            d_out_p = d_out[:].rearrange("(bh p t) v -> t p bh v", t=T, p=P)

            osb_open = {}

            def proj_chunk(tp, y2g_src, piece):
                """One (batch-half, vocab-chunk) piece of step tp's
                projection: 8 matmuls into a double-buffered PSUM bank + one
                ACT drain; the store fires on the last piece."""
                bh, n = piece
                if tp not in osb_open:
                    osb_open[tp] = outp.tile([P, 2, VS], F32, tag="osb",
                                             name=f"osb_{tp}")
                o_sb = osb_open[tp]
                noff, nlen = NCHUNKS[n]
                pp = psumP.tile([P, 512], F32, tag="pp",
                                name=f"pp_{tp}_{bh}_{n}")
                for k in range(NKH):
                    nc.tensor.matmul(
                        pp[:, 0:nlen],
                        y2g_src[:, k, bh * P:(bh + 1) * P],
                        w_out[:, k, noff:noff + nlen],
                        start=(k == 0), stop=(k == NKH - 1))
                nc.scalar.activation(
                    o_sb[:, bh, noff:noff + nlen], pp[:, 0:nlen],
                    mybir.ActivationFunctionType.Identity)
                if bh == 1 and n == len(NCHUNKS) - 1:
                    dma_act(d_out_p[tp], o_sb[:])
                    del osb_open[tp]

            PIECES = [(0, 0), (0, 1), (0, 2), (1, 0), (1, 1), (1, 2)]

            def stage(idx, hy, lo, hi):
                agir = agi[idx % RING][:].rearrange("s p b -> p s b")
                dma_sp(agir[:, lo:hi], hy[:])

            def all_gather(idx, dsts):
                """AG on ring slot idx; dsts: (queue, dst_ap, slot, klo, khi)
                readback requests, issued in order per queue."""
                ai = agi[idx % RING]
                ao = ago[idx % RING]
                if FAKE_AG:
                    dma_sp(ao[0][:], ai[:])
                else:
                    nc.gpsimd.collective_compute(
                        "AllGather", mybir.AluOpType.bypass,
                        ins=[ai[:].opt()], outs=[ao[:].opt()],
                        replica_groups=[list(range(NCORE))])
                agor = ao[:].rearrange("c s p b -> s p c b")
                for queue, dst, slot, klo, khi in dsts:
                    queue(dst, agor[slot, :, klo:khi])

            NH = NKH // 2

            pending_l1 = emit_ih1(0, x_t, stop=True)
            y2g_prev = None    # y2(t-2) gathered, for proj(t-2) in iter t
            y2g_cur = None     # y2(t-1) gathered during iter t
            h1g = None

            for t in range(T):
                # ---- cell1(t): psums completed last iteration ----
                hy1 = slpool.tile([P, 2, B], dt, tag="hy1", name=f"hy1_{t}")
                lstm_cell(b1, gb1, m1, c1, pending_l1, hy1, f"1_{t}")
                stage(t, hy1, 0, 2)

                # ---- the step's single AllGather ----
                y1g = gp_y1.tile([P, NKH, B], dt, tag="y1g", name=f"y1g_{t}")
                h2g = (gp_h2.tile([P, NKH, B], dt, tag="h2g", name=f"h2g_{t}")
                       if t > 0 else None)
                dsts = [(dma_sp, y1g[:, 0:NH], 1, 0, NH),
                        (dma_sp, y1g[:, NH:], 1, NH, NKH)]
                if t > 0:
                    dsts += [(dma_dve, h2g[:, 0:NH], 2, 0, NH),
                             (dma_dve, h2g[:, NH:], 2, NH, NKH)]
                if t + 1 < T:
                    h1g = gp_h1.tile([P, NKH, B], dt, tag="h1g",
                                     name=f"h1g_{t}")
                    dsts.append((dma_sp, h1g[:], 0, 0, NKH))
                if t > 0:
                    y2g_cur = gp_y2.tile([P, NKH, B], dt, tag="y2g",
                                         name=f"y2g_{t}")
                    dsts.append((dma_dve, y2g_cur[:], 3, 0, NKH))
                all_gather(t, dsts)
                # x(t+1) load rides ACT behind the h2g/y2g readbacks
                x_nxt = load_x(t + 1) if t + 1 < T else None

                # ---- PE filler while the AG flies (all AG-independent) ----
                if y2g_prev is not None:
                    proj_chunk(t - 2, y2g_prev, PIECES[0])
                    proj_chunk(t - 2, y2g_prev, PIECES[1])
                if t + 1 < T:
                    pending_l1 = emit_ih1(t + 1, x_nxt, stop=False)
                if y2g_prev is not None:
                    proj_chunk(t - 2, y2g_prev, PIECES[2])
                    proj_chunk(t - 2, y2g_prev, PIECES[3])

                # ---- layer 2 (waits on the AG readbacks) ----
                pending_l2 = emit_l2(t, y1g, h2g)
                hy2 = slpool.tile([P, 2, B], dt, tag="hy2", name=f"hy2_{t}")
                lstm_cell(b2, gb2, m2, c2, pending_l2, hy2, f"2_{t}")
                stage(t + 1, hy2, 2, 4)

                # ---- layer-1 recurrence for t+1 + trailing proj filler ----
                if t + 1 < T:
                    emit_hh1(pending_l1, h1g)
                if y2g_prev is not None:
                    proj_chunk(t - 2, y2g_prev, PIECES[4])
                    proj_chunk(t - 2, y2g_prev, PIECES[5])
                y2g_prev = y2g_cur

            # ---- epilogue: gather h2/y2(T-1); project steps T-2, T-1 ----
            y2g_last = gp_y2.tile([P, NKH, B], dt, tag="y2g", name="y2g_T")
            all_gather(T, [(dma_dve, y2g_last[:], 3, 0, NKH)])
            for piece in PIECES:                       # proj(T-2): y2g landed
                proj_chunk(T - 2, y2g_prev, piece)     # with AG(T-1)
            for piece in PIECES:
                proj_chunk(T - 1, y2g_last, piece)

    nc.finalize()
    return nc


def _prep_inputs(features, captions, lengths, embed_table,
                 W_ih1, W_hh1, b_ih1, b_hh1, gamma1, beta1, mask1,
                 W_ih2, W_hh2, b_ih2, b_hh2, gamma2, beta2, mask2,
                 W_out, b_out):
    f32 = np.float32
    features = np.asarray(features, f32)
    captions = np.asarray(captions)
    embed_table = np.asarray(embed_table, f32)
    if DT_MM == F32R:
        rnd = _fp32r_round
    elif DT_MM == BF16:
        rnd = lambda a: np.ascontiguousarray(a.astype(ml_dtypes.bfloat16))
    else:
        rnd = lambda a: a

    # x sequence [T, B, E] -> xT [T, NKE, P, B]
    x = np.empty((L + 1, B, E), f32)
    x[0] = features
    x[1:] = embed_table[captions].transpose(1, 0, 2)
    x = x[:T]
    xT = rnd(np.ascontiguousarray(x.transpose(0, 2, 1).reshape(T, NKE, P, B)))

    def wslice(Wf, c, K):
        # Wf [4H, K] -> per-core [4, K//P, P, HS] lhsT blocks
        Wg = np.asarray(Wf, f32).reshape(4, H, K)[:, c * HS:(c + 1) * HS, :]
        return rnd(np.ascontiguousarray(
            Wg.transpose(0, 2, 1).reshape(4, K // P, P, HS)))

    bsum1 = (np.asarray(b_ih1, f32) + np.asarray(b_hh1, f32)).reshape(4, H)
    bsum2 = (np.asarray(b_ih2, f32) + np.asarray(b_hh2, f32)).reshape(4, H)
    WoT = np.ascontiguousarray(np.asarray(W_out, f32).T)  # [H, V]

    in_maps = []
    for c in range(NCORE):
        u = slice(c * HS, (c + 1) * HS)
        v = slice(c * VS, (c + 1) * VS)
        in_maps.append({
            "xT": xT,
            "wih1": wslice(W_ih1, c, E),
            "whh1": wslice(W_hh1, c, H),
            "wih2": wslice(W_ih2, c, H),
            "whh2": wslice(W_hh2, c, H),
            "woutT": rnd(np.ascontiguousarray(
                WoT[:, v].reshape(NKH, P, VS))),
            "bias1": np.ascontiguousarray(bsum1[:, u].T),
            "bias2": np.ascontiguousarray(bsum2[:, u].T),
            "gb1": np.ascontiguousarray(
                np.stack([np.asarray(gamma1, f32)[u],
                          np.asarray(beta1, f32)[u]], axis=1)),
            "gb2": np.ascontiguousarray(
                np.stack([np.asarray(gamma2, f32)[u],
                          np.asarray(beta2, f32)[u]], axis=1)),
            "m1T": np.ascontiguousarray(np.asarray(mask1, f32).T[u]),
            "m2T": np.ascontiguousarray(np.asarray(mask2, f32).T[u]),
        })
    return in_maps, np.asarray(b_out, f32)


def kernel(**inputs):
    global LAST_EXEC_NS
    if "nc" not in _CACHE:
        _CACHE["nc"] = build_bass()
    nc = _CACHE["nc"]

    in_maps, b_out = _prep_inputs(**inputs)
    trace = os.environ.get("TRN_KERNEL_TRACE", "0") == "1"
    res = run_bass_kernel_spmd(nc, in_maps, core_ids=list(range(NCORE)),
                               trace=trace)
    LAST_EXEC_NS = res.exec_time_ns
    out = np.concatenate([res.results[c]["out"] for c in range(NCORE)], axis=1)
    if b_out.any():
        out = out + b_out[None, :]
    return out


# revision 12
# speedup vs baseline: 1.1416x; 1.1416x over previous
"""Trainium2 Bass kernel for nn_Decoder (2-layer LSTM decoder with BatchNorm +
LockedDropout + vocab projection), tensor-parallel over the hidden dim across
8 NeuronCores.

Contract: kernel(**inputs) takes FULL inputs (as produced by setup_inputs())
and returns the FULL [B*T, V] float32 output.

Sharding:
  - Each core owns a 128-unit slice of the hidden dim for BOTH LSTM layers
    (gates i,f,g,o for those units) -> gate matmuls have M=128 per gate with
    full batch B=256 as the moving dim (full PE width, BN stats exact over
    the full batch).
  - ONE AllGather per step (ncfw, through HBM) carries two [P, B+2] rows:
    [h1(t) | its BN mean,var] staged by this step's cell1, and
    [h2(t-1) | stats] staged at the end of the previous step.  Only raw h
    and 2 stats columns travel: every consumer derives the BatchNorm'd +
    masked y itself (an ACT affine per 128-tile using per-partition
    scale/bias from the gathered stats, then a Pool-engine mask multiply).
    This keeps the pre-collective chain to cast+bn_stats (~1us after the
    cell's last gate), halves the collective payload, and drops the
    post-collective readback to two 520KB DMAs.
  - The vocab projection is sharded over V (1250 per core).

Schedule (per step t) -- emission order is the Tile scheduler's priority:
  cell1(t) -> stage [h1|stats ; h2(t-1)|stats] (one DMA) -> AG(t) ->
  readback h1x(t), h2x(t-1) -> [PE filler while the AG flies: proj(t-2)
  chunks + ih1(t+1) pre-emit] -> y1 derivation (DVE rsqrt chain + 8 ACT
  affines paced with the PE) -> ih2 (k-major, consuming y1 tiles as they
  are produced) -> hh2/hh1 (gate-major so the cells' ACT chains start
  early) -> cell2(t) -> y2(t-1) derivation (for next step's projection)
  -> trailing proj chunks.

PSUM: 6 banks rotate over the LSTM gate groups (layer1(t), layer2(t),
layer1(t+1) pre-emitted), one accumulation group per bank (start on the
bank's first matmul, gates 0/1 only); 2 banks double-buffer the projection.

Matmul/transport dtype is bf16: full PE rate, half the collective/DMA
bytes; BN stats ride in bf16 (the ~0.4% stats rounding is below the bf16
y-rounding the previous y-transport scheme already had).
"""

import contextlib
import os
import sys

sys.path.insert(0, "/opt/trn_rl_repo")

import ml_dtypes
import numpy as np

import concourse.bass as bass
import concourse.tile as tile
from concourse import bacc, mybir
from concourse.bass_utils import run_bass_kernel_spmd

F32 = mybir.dt.float32
F32R = mybir.dt.float32r
BF16 = mybir.dt.bfloat16

DT_MM_NAME = os.environ.get("TRN_DT_MM", "bf16")
DT_MM = {"f32r": F32R, "bf16": BF16, "f32": F32}[DT_MM_NAME]

B, L, E, H, V = 256, 20, 512, 1024, 10000
T = int(os.environ.get("TRN_T", L + 1))
NCORE = 8
P = 128
HS = H // NCORE          # 128 hidden units per core per layer
VS = V // NCORE          # 1250 vocab slots per core
NKE = E // P             # 4 k-tiles over E
NKH = H // P             # 8 k-tiles over H
BX = B + 2               # gathered row: [h (B) | mean | var]
BN_EPS = 1e-5
# projection N-chunks of VS=1250
NCHUNKS = [(0, 418), (418, 416), (834, 416)]

LAST_EXEC_NS = None
# TRN_FAKE_AG=1 replaces collectives with a local DMA (debug-only)
FAKE_AG = os.environ.get("TRN_FAKE_AG", "0") == "1"

_CACHE = {}


def _fp32r_round(x):
    hi = x.astype(ml_dtypes.bfloat16).astype(np.float32)
    lo = (x - hi).astype(ml_dtypes.bfloat16).astype(np.float32)
    return hi + lo


def build_bass():
    nc = bacc.Bacc("TRN2", target_bir_lowering=False, num_devices=NCORE)
    dt = DT_MM
    ddt = dt

    # ---------------- DRAM I/O ----------------
    d_xT = nc.dram_tensor("xT", [T, NKE, P, B], ddt, kind="ExternalInput")
    d_wih1 = nc.dram_tensor("wih1", [4, NKE, P, HS], ddt, kind="ExternalInput")
    d_whh1 = nc.dram_tensor("whh1", [4, NKH, P, HS], ddt, kind="ExternalInput")
    d_wih2 = nc.dram_tensor("wih2", [4, NKH, P, HS], ddt, kind="ExternalInput")
    d_whh2 = nc.dram_tensor("whh2", [4, NKH, P, HS], ddt, kind="ExternalInput")
    d_woutT = nc.dram_tensor("woutT", [NKH, P, VS], ddt, kind="ExternalInput")
    d_bias1 = nc.dram_tensor("bias1", [HS, 4], F32, kind="ExternalInput")
    d_bias2 = nc.dram_tensor("bias2", [HS, 4], F32, kind="ExternalInput")
    # full-H BN gamma/beta [P, NKH, 2] and full-H masks [NKH, P, B] (bf16):
    # every core derives y for all 1024 units from the gathered h + stats.
    d_gb1 = nc.dram_tensor("gb1", [P, NKH, 2], F32, kind="ExternalInput")
    d_gb2 = nc.dram_tensor("gb2", [P, NKH, 2], F32, kind="ExternalInput")
    d_m1T = nc.dram_tensor("m1T", [NKH, P, B], ddt, kind="ExternalInput")
    d_m2T = nc.dram_tensor("m2T", [NKH, P, B], ddt, kind="ExternalInput")
    d_out = nc.dram_tensor("out", [B * T, VS], F32, kind="ExternalOutput")

    # collective bounce buffers (inputs must be Local, outputs Shared)
    # slot layout: 0 = [h1(t)|stats], 1 = [h2(t-1)|stats]
    RING = 3
    agi = [nc.dram_tensor(f"agi{j}", [2, P, BX], dt, kind="Internal")
           for j in range(RING)]
    ago = [nc.dram_tensor(f"ago{j}", [NCORE, 2, P, BX], dt,
                          kind="Internal", addr_space="Shared")
           for j in range(RING)]

    # SP ring: latency-critical stage + gather readbacks.
    # ACT ring: weights, x loads, projection stores.
    dma_sp = nc.sync.dma_start
    dma_act = nc.scalar.dma_start

    with tile.TileContext(nc) as tc:
        with contextlib.ExitStack() as ctx:
            smalls = ctx.enter_context(tc.tile_pool(name="smalls", bufs=1))
            wts = ctx.enter_context(tc.tile_pool(name="wts", bufs=1))
            xpool = ctx.enter_context(tc.tile_pool(name="xpool", bufs=3))
            gp_h1 = ctx.enter_context(tc.tile_pool(name="g_h1", bufs=2))
            gp_h2 = ctx.enter_context(tc.tile_pool(name="g_h2", bufs=2))
            gp_y1 = ctx.enter_context(tc.tile_pool(name="g_y1", bufs=2))
            gp_y2 = ctx.enter_context(tc.tile_pool(name="g_y2", bufs=2))
            cell = ctx.enter_context(tc.tile_pool(name="cell", bufs=2))
            slpool = ctx.enter_context(tc.tile_pool(name="slp", bufs=3))
            state = ctx.enter_context(tc.tile_pool(name="state", bufs=1))
            psumg = ctx.enter_context(
                tc.tile_pool(name="psumg", bufs=6, space="PSUM"))
            psumP = ctx.enter_context(
                tc.tile_pool(name="psumP", bufs=2, space="PSUM"))
            outp = ctx.enter_context(tc.tile_pool(name="outp", bufs=3))

            # resident weights
            w_ih1 = wts.tile([P, 4, NKE, HS], dt)
            w_hh1 = wts.tile([P, 4, NKH, HS], dt)
            w_ih2 = wts.tile([P, 4, NKH, HS], dt)
            w_hh2 = wts.tile([P, 4, NKH, HS], dt)
            w_out = wts.tile([P, NKH, VS], dt)

            def load_weight(dstq, dst, dram):
                dstq(dst[:], dram[:].rearrange("g k p m -> p g k m"))

            def load_x(t):
                x_t = xpool.tile([P, NKE, B], dt, tag="x", name=f"x_{t}")
                dma_act(x_t[:], d_xT[t][:].rearrange("k p b -> p k b"))
                return x_t

            # prologue: wih1 + x0 unblock the PE first (split across both
            # rings), then the remaining weights stream on ACT in first-use
            # order while the smalls ride SP.
            load_weight(dma_sp, w_ih1, d_wih1)
            x_t = load_x(0)
            load_weight(dma_act, w_ih2, d_wih2)
            load_weight(dma_act, w_hh1, d_whh1)
            load_weight(dma_act, w_hh2, d_whh2)
            dma_act(w_out[:], d_woutT[:].rearrange("k p v -> p k v"))

            b1 = smalls.tile([HS, 4], F32)
            b2 = smalls.tile([HS, 4], F32)
            gb1 = smalls.tile([P, NKH, 2], F32)
            gb2 = smalls.tile([P, NKH, 2], F32)
            m1 = smalls.tile([P, NKH, B], dt)
            m2 = smalls.tile([P, NKH, B], dt)
            for dst, src in ((b1, d_bias1), (b2, d_bias2),
                             (gb1, d_gb1), (gb2, d_gb2)):
                dma_sp(dst[:], src[:])
            dma_sp(m1[:], d_m1T[:].rearrange("k p b -> p k b"))
            dma_sp(m2[:], d_m2T[:].rearrange("k p b -> p k b"))

            # persistent state
            c1 = state.tile([P, B], F32)
            c2 = state.tile([P, B], F32)
            nc.vector.memset(c1[:], 0.0)
            nc.vector.memset(c2[:], 0.0)

            def lstm_cell(bias, c_st, pgs, hs_out, s, t):
                """One LSTM cell; writes [h-cast | bn mean,var] into
                hs_out[:, s] (a [P, BX] bf16 staging row).

                psum packing: pgA=(i,g), pgB=(f,o); gate order i=0 f=1 g=2 o=3.
                """
                pgA, pgB = pgs
                i_t = cell.tile([P, B], F32, tag="i", name=f"i_{t}")
                f_t = cell.tile([P, B], F32, tag="f", name=f"f_{t}")
                g_t = cell.tile([P, B], F32, tag="g", name=f"g_{t}")
                o_t = cell.tile([P, B], F32, tag="o", name=f"o_{t}")
                Sig = mybir.ActivationFunctionType.Sigmoid
                Tanh = mybir.ActivationFunctionType.Tanh
                nc.scalar.activation(i_t[:], pgA[:, 0], Sig, bias=bias[:, 0:1])
                nc.scalar.activation(g_t[:], pgA[:, 1], Tanh, bias=bias[:, 2:3])
                nc.scalar.activation(f_t[:], pgB[:, 0], Sig, bias=bias[:, 1:2])
                nc.scalar.activation(o_t[:], pgB[:, 1], Sig, bias=bias[:, 3:4])

                ig = cell.tile([P, B], F32, tag="ig", name=f"ig_{t}")
                nc.vector.tensor_mul(ig[:], i_t[:], g_t[:])
                fc = cell.tile([P, B], F32, tag="fc", name=f"fc_{t}")
                nc.vector.tensor_mul(fc[:], f_t[:], c_st[:])
                nc.vector.tensor_add(c_st[:], ig[:], fc[:])
                tnc = cell.tile([P, B], F32, tag="tc", name=f"tc_{t}")
                nc.scalar.activation(tnc[:], c_st[:], Tanh)
                h_f = cell.tile([P, B], F32, tag="h", name=f"h_{t}")
                nc.vector.tensor_mul(h_f[:], o_t[:], tnc[:])
                nc.scalar.activation(hs_out[:, s, 0:B], h_f[:],
                                     mybir.ActivationFunctionType.Identity)
                # BN stats over batch (free dim); mean/var ride the collective
                st6 = cell.tile([P, 6], F32, tag="st", name=f"st_{t}")
                nc.vector.bn_stats(st6[:], h_f[:])
                mv = cell.tile([P, 2], F32, tag="mv", name=f"mv_{t}")
                nc.vector.bn_aggr(mv[:], st6[:])
                nc.vector.tensor_copy(hs_out[:, s, B:BX], mv[:])

            I32 = mybir.dt.int32

            def bn_derive(hgx, gbv, mask, ypool, tag, t):
                """Derive y = BN(h)*mask for all NKH tiles from a gathered
                [P, NKH, BX] row-block.  rsqrt via fast-inverse-sqrt + one
                Newton step, batched over all tiles on the DVE; per-tile ACT
                affine (per-partition scale/bias) + Pool-engine mask mul."""
                mean = hgx[:, :, B:B + 1]           # [P, NKH, 1] bf16 views
                var = hgx[:, :, B + 1:BX]
                v_t = cell.tile([P, NKH, 1], F32, tag="vv", name=f"vv_{t}")
                nc.vector.tensor_scalar_add(v_t[:], var, BN_EPS)
                r_a = cell.tile([P, NKH, 1], F32, tag="ra", name=f"ra_{t}")
                ui = cell.tile([P, NKH, 1], I32, tag="ui", name=f"ui_{t}")
                nc.vector.tensor_scalar(ui[:], v_t[:].bitcast(I32), 1, None,
                                        op0=mybir.AluOpType.logical_shift_right)
                nc.vector.tensor_scalar(r_a[:].bitcast(I32), ui[:],
                                        -1, 0x5F3759DF,
                                        op0=mybir.AluOpType.mult,
                                        op1=mybir.AluOpType.add)
                rr = cell.tile([P, NKH, 1], F32, tag="rr", name=f"rr_{t}")
                ww = cell.tile([P, NKH, 1], F32, tag="ww", name=f"ww_{t}")
                r_b = cell.tile([P, NKH, 1], F32, tag="rb", name=f"rb_{t}")
                nc.vector.tensor_mul(rr[:], r_a[:], r_a[:])
                nc.vector.scalar_tensor_tensor(
                    ww[:], rr[:], -0.5, v_t[:],
                    op0=mybir.AluOpType.mult, op1=mybir.AluOpType.mult)
                nc.vector.scalar_tensor_tensor(
                    r_b[:], ww[:], 1.5, r_a[:],
                    op0=mybir.AluOpType.add, op1=mybir.AluOpType.mult)
                a_v = cell.tile([P, NKH, 1], F32, tag="av", name=f"av_{t}")
                nc.vector.tensor_mul(a_v[:], r_b[:], gbv[:, :, 0:1])
                ma = cell.tile([P, NKH, 1], F32, tag="ma", name=f"ma_{t}")
                nc.vector.tensor_mul(ma[:], mean, a_v[:])
                b_v = cell.tile([P, NKH, 1], F32, tag="bv", name=f"bv_{t}")
                nc.vector.tensor_sub(b_v[:], gbv[:, :, 1:2], ma[:])
                y = ypool.tile([P, NKH, B], dt, tag=tag, name=f"{tag}_{t}")
                for k in range(NKH):
                    nc.scalar.activation(
                        y[:, k], hgx[:, k, 0:B],
                        mybir.ActivationFunctionType.Identity,
                        bias=b_v[:, k], scale=a_v[:, k])
                    nc.gpsimd.tensor_mul(y[:, k], y[:, k], mask[:, k])
                return y

            def emit_ih1(t, x_t, stop):
                # x-side of layer-1 gates for step t.  One accumulation group
                # per PSUM bank: start on the bank's first matmul (gates 0/1),
                # stop on each gate's last matmul (deferred to the hh matmuls
                # unless `stop`).
                pgA = psumg.tile([P, 2, B], F32, tag="pg", name=f"pgA_1_{t}")
                pgB = psumg.tile([P, 2, B], F32, tag="pg", name=f"pgB_1_{t}")
                gloc = {0: (pgA, 0), 2: (pgA, 1), 1: (pgB, 0), 3: (pgB, 1)}
                for gate in (0, 2, 1, 3):
                    tl, sub = gloc[gate]
                    for k in range(NKE):
                        nc.tensor.matmul(
                            tl[:, sub], w_ih1[:, gate, k], x_t[:, k],
                            start=(k == 0 and gate in (0, 1)),
                            stop=(stop and k == NKE - 1 and gate in (2, 3)))
                return pgA, pgB

            def emit_ih2(t, y1g, stop):
                # layer-2 x-side, k-major: consumes y1 tile k for ~0.43us per
                # k, pacing the ACT affines that produce them.
                pgA = psumg.tile([P, 2, B], F32, tag="pg", name=f"pgA_2_{t}")
                pgB = psumg.tile([P, 2, B], F32, tag="pg", name=f"pgB_2_{t}")
                gloc = {0: (pgA, 0), 2: (pgA, 1), 1: (pgB, 0), 3: (pgB, 1)}
                last = NKH - 1
                for k in range(NKH):
                    for gate in (0, 2, 1, 3):
                        tl, sub = gloc[gate]
                        nc.tensor.matmul(
                            tl[:, sub], w_ih2[:, gate, k], y1g[:, k],
                            start=(k == 0 and gate in (0, 1)),
                            stop=(stop and k == last))
                return pgA, pgB

            def emit_hh(pgs, w, hg, widx):
                # recurrent half for either layer, gate-major so each gate's
                # accumulation completes every 8 matmuls and the cell's ACT
                # chain starts early.  hg rows are [P, BX]; h is cols 0:B.
                pgA, pgB = pgs
                gloc = {0: (pgA, 0), 2: (pgA, 1), 1: (pgB, 0), 3: (pgB, 1)}
                for gate in (0, 2, 1, 3):
                    tl, sub = gloc[gate]
                    for k in range(NKH):
                        nc.tensor.matmul(
                            tl[:, sub], w[:, gate, k], hg[:, k, 0:B],
                            start=False, stop=(k == NKH - 1))

            # out rows at step tp as [p, bh, v] (partition-major SBUF view)
            d_out_p = d_out[:].rearrange("(bh p t) v -> t p bh v", t=T, p=P)

            osb_open = {}

            def proj_chunk(tp, y2g_src, piece):
                """One (batch-half, vocab-chunk) piece of step tp's
                projection: 8 matmuls into a double-buffered PSUM bank + one
                drain; the store fires on the last piece.  Drains alternate
                ACT/DVE to keep the ACT ring free for the cell chains."""
                bh, n = piece
                if tp not in osb_open:
                    osb_open[tp] = outp.tile([P, 2, VS], F32, tag="osb",
                                             name=f"osb_{tp}")
                o_sb = osb_open[tp]
                noff, nlen = NCHUNKS[n]
                pp = psumP.tile([P, 512], F32, tag="pp",
                                name=f"pp_{tp}_{bh}_{n}")
                for k in range(NKH):
                    nc.tensor.matmul(
                        pp[:, 0:nlen],
                        y2g_src[:, k, bh * P:(bh + 1) * P],
                        w_out[:, k, noff:noff + nlen],
                        start=(k == 0), stop=(k == NKH - 1))
                if n % 2 == 0:
                    nc.scalar.activation(
                        o_sb[:, bh, noff:noff + nlen], pp[:, 0:nlen],
                        mybir.ActivationFunctionType.Identity)
                else:
                    nc.vector.tensor_copy(
                        o_sb[:, bh, noff:noff + nlen], pp[:, 0:nlen])
                if bh == 1 and n == len(NCHUNKS) - 1:
                    dma_act(d_out_p[tp], o_sb[:])
                    del osb_open[tp]

            PIECES = [(0, 0), (0, 1), (0, 2), (1, 0), (1, 1), (1, 2)]

            def all_gather(idx, dsts):
                """AG on ring slot idx; dsts: (dst_ap, slot) full-row
                readbacks on the SP ring."""
                ai = agi[idx % RING]
                ao = ago[idx % RING]
                if FAKE_AG:
                    dma_sp(ao[0][:], ai[:])
                else:
                    nc.gpsimd.collective_compute(
                        "AllGather", mybir.AluOpType.bypass,
                        ins=[ai[:].opt()], outs=[ao[:].opt()],
                        replica_groups=[list(range(NCORE))])
                agor = ao[:].rearrange("c s p x -> s p c x")
                for dst, slot in dsts:
                    dma_sp(dst, agor[slot])

            pending_l1 = emit_ih1(0, x_t, stop=True)
            y2g_prev = None    # y2(t-2), for proj(t-2) in iter t
            y2g_cur = None     # y2(t-1), derived during iter t
            hs_cur = slpool.tile([P, 2, BX], dt, tag="hs", name="hs_0")
            nc.vector.memset(hs_cur[:, 1], 0.0)   # h2(-1) | stats = 0

            for t in range(T):
                # ---- cell1(t): psums completed last iteration ----
                lstm_cell(b1, c1, pending_l1, hs_cur, 0, f"1_{t}")
                agir = agi[t % RING][:].rearrange("s p x -> p s x")
                dma_sp(agir[:], hs_cur[:])

                # ---- the step's single AllGather + readbacks ----
                h1gx = gp_h1.tile([P, NKH, BX], dt, tag="h1x", name=f"h1x_{t}")
                dsts = [(h1gx[:], 0)]
                h2gx = None
                if t > 0:
                    h2gx = gp_h2.tile([P, NKH, BX], dt, tag="h2x",
                                      name=f"h2x_{t}")
                    dsts.append((h2gx[:], 1))
                all_gather(t, dsts)
                x_nxt = load_x(t + 1) if t + 1 < T else None

                # ---- PE filler while the AG flies (all AG-independent) ----
                if y2g_prev is not None:
                    proj_chunk(t - 2, y2g_prev, PIECES[0])
                    proj_chunk(t - 2, y2g_prev, PIECES[1])
                if t + 1 < T:
                    pending_l1 = emit_ih1(t + 1, x_nxt, stop=False)
                if y2g_prev is not None:
                    proj_chunk(t - 2, y2g_prev, PIECES[2])
                    proj_chunk(t - 2, y2g_prev, PIECES[3])

                # ---- derive y1(t) from the gathered h1+stats, feed L2 ----
                y1g = bn_derive(h1gx, gb1, m1, gp_y1, "y1g", t)
                pending_l2 = emit_ih2(t, y1g, stop=(t == 0))
                if t > 0:
                    emit_hh(pending_l2, w_hh2, h2gx, 2)
                if t + 1 < T:
                    emit_hh(pending_l1, w_hh1, h1gx, 1)

                # ---- cell2(t) stages into next step's collective ----
                hs_nxt = slpool.tile([P, 2, BX], dt, tag="hs",
                                     name=f"hs_{t+1}")
                lstm_cell(b2, c2, pending_l2, hs_nxt, 1, f"2_{t}")
                hs_cur = hs_nxt

                # ---- trailing proj filler + y2(t-1) for the next window ----
                if y2g_prev is not None:
                    proj_chunk(t - 2, y2g_prev, PIECES[4])
                    proj_chunk(t - 2, y2g_prev, PIECES[5])
                if t > 0:
                    y2g_cur = bn_derive(h2gx, gb2, m2, gp_y2, "y2g", t)
                y2g_prev = y2g_cur

            # ---- epilogue: gather h2(T-1); project steps T-2, T-1 ----
            h2gx_last = gp_h2.tile([P, NKH, BX], dt, tag="h2x", name="h2x_T")
            nc.vector.memset(hs_cur[:, 0], 0.0)   # no cell1(T); keep finite
            agir = agi[T % RING][:].rearrange("s p x -> p s x")
            dma_sp(agir[:], hs_cur[:])
            all_gather(T, [(h2gx_last[:], 1)])
            for piece in PIECES:                     # proj(T-2): y2 derived
                proj_chunk(T - 2, y2g_prev, piece)   # during iter T-1
            y2g_last = bn_derive(h2gx_last, gb2, m2, gp_y2, "y2g", T)
            for piece in PIECES:
                proj_chunk(T - 1, y2g_last, piece)

    nc.finalize()
    return nc


def _prep_inputs(features, captions, lengths, embed_table,
                 W_ih1, W_hh1, b_ih1, b_hh1, gamma1, beta1, mask1,
                 W_ih2, W_hh2, b_ih2, b_hh2, gamma2, beta2, mask2,
                 W_out, b_out):
    f32 = np.float32
    features = np.asarray(features, f32)
    captions = np.asarray(captions)
    embed_table = np.asarray(embed_table, f32)
    if DT_MM == F32R:
        rnd = _fp32r_round
    elif DT_MM == BF16:
        rnd = lambda a: np.ascontiguousarray(a.astype(ml_dtypes.bfloat16))
    else:
        rnd = lambda a: a

    # x sequence [T, B, E] -> xT [T, NKE, P, B]
    x = np.empty((L + 1, B, E), f32)
    x[0] = features
    x[1:] = embed_table[captions].transpose(1, 0, 2)
    x = x[:T]
    xT = rnd(np.ascontiguousarray(x.transpose(0, 2, 1).reshape(T, NKE, P, B)))

    def wslice(Wf, c, K):
        # Wf [4H, K] -> per-core [4, K//P, P, HS] lhsT blocks
        Wg = np.asarray(Wf, f32).reshape(4, H, K)[:, c * HS:(c + 1) * HS, :]
        return rnd(np.ascontiguousarray(
            Wg.transpose(0, 2, 1).reshape(4, K // P, P, HS)))

    def gbfull(gamma, beta):
        # [H] pair -> [P, NKH, 2] (tile k, partition p) = unit k*P+p
        g = np.asarray(gamma, f32).reshape(NKH, P).T
        b = np.asarray(beta, f32).reshape(NKH, P).T
        return np.ascontiguousarray(np.stack([g, b], axis=2))

    def maskfull(mask):
        # [B, H] -> [NKH, P, B] bf16
        mT = np.asarray(mask, f32).T.reshape(NKH, P, B)
        return rnd(np.ascontiguousarray(mT))

    bsum1 = (np.asarray(b_ih1, f32) + np.asarray(b_hh1, f32)).reshape(4, H)
    bsum2 = (np.asarray(b_ih2, f32) + np.asarray(b_hh2, f32)).reshape(4, H)
    WoT = np.ascontiguousarray(np.asarray(W_out, f32).T)  # [H, V]

    gb1v, gb2v = gbfull(gamma1, beta1), gbfull(gamma2, beta2)
    m1v, m2v = maskfull(mask1), maskfull(mask2)

    in_maps = []
    for c in range(NCORE):
        u = slice(c * HS, (c + 1) * HS)
        v = slice(c * VS, (c + 1) * VS)
        in_maps.append({
            "xT": xT,
            "wih1": wslice(W_ih1, c, E),
            "whh1": wslice(W_hh1, c, H),
            "wih2": wslice(W_ih2, c, H),
            "whh2": wslice(W_hh2, c, H),
            "woutT": rnd(np.ascontiguousarray(
                WoT[:, v].reshape(NKH, P, VS))),
            "bias1": np.ascontiguousarray(bsum1[:, u].T),
            "bias2": np.ascontiguousarray(bsum2[:, u].T),
            "gb1": gb1v,
            "gb2": gb2v,
            "m1T": m1v,
            "m2T": m2v,
        })
    return in_maps, np.asarray(b_out, f32)


def kernel(**inputs):
    global LAST_EXEC_NS
    if "nc" not in _CACHE:
        _CACHE["nc"] = build_bass()
    nc = _CACHE["nc"]

    in_maps, b_out = _prep_inputs(**inputs)
    trace = os.environ.get("TRN_KERNEL_TRACE", "0") == "1"
    res = run_bass_kernel_spmd(nc, in_maps, core_ids=list(range(NCORE)),
                               trace=trace)
    LAST_EXEC_NS = res.exec_time_ns
    out = np.concatenate([res.results[c]["out"] for c in range(NCORE)], axis=1)
    if b_out.any():
        out = out + b_out[None, :]
    return out


# revision 25
# speedup vs baseline: 1.4936x; 1.3083x over previous
"""Trainium2 Bass kernel for nn_Decoder (2-layer LSTM decoder with BatchNorm +
LockedDropout + vocab projection), tensor-parallel over the hidden dim across
8 NeuronCores.

Contract: kernel(**inputs) takes FULL inputs (as produced by setup_inputs())
and returns the FULL [B*T, V] float32 output.

Sharding:
  - Each core owns a 128-unit slice of the hidden dim for BOTH LSTM layers
    (gates i,f,g,o for those units) -> gate matmuls have M=128 per gate with
    full batch B=256 as the moving dim (full PE width, BN stats exact over
    the full batch).
  - ONE AllGather per step (ncfw, through HBM) carries two [P, B+2] rows:
    [h1(t) | its BN mean,var] staged by this step's cell1, and
    [h2(t-1) | stats] staged at the end of the previous step.  Only raw h
    and 2 stats columns travel: every consumer derives the BatchNorm'd +
    masked y itself (an ACT affine per 128-tile using per-partition
    scale/bias from the gathered stats, then a Pool-engine mask multiply).
    This keeps the pre-collective chain to cast+bn_stats (~1us after the
    cell's last gate), halves the collective payload, and drops the
    post-collective readback to two 520KB DMAs.
  - The vocab projection is sharded over V (1250 per core).

Schedule (per step t) -- emission order is the Tile scheduler's priority:
  cell1(t) -> stage [h1|stats ; h2(t-1)|stats] (one DMA) -> AG(t) ->
  readback h1x(t), h2x(t-1) -> [PE filler while the AG flies: proj(t-2)
  chunks + ih1(t+1) pre-emit] -> y1 derivation (DVE rsqrt chain + 8 ACT
  affines paced with the PE) -> ih2 (k-major, consuming y1 tiles as they
  are produced) -> hh2/hh1 (gate-major so the cells' ACT chains start
  early) -> cell2(t) -> y2(t-1) derivation (for next step's projection)
  -> trailing proj chunks.

PSUM: 6 banks rotate over the LSTM gate groups (layer1(t), layer2(t),
layer1(t+1) pre-emitted), one accumulation group per bank (start on the
bank's first matmul, gates 0/1 only); 2 banks double-buffer the projection.

Matmul/transport dtype is bf16: full PE rate, half the collective/DMA
bytes; BN stats ride in bf16 (the ~0.4% stats rounding is below the bf16
y-rounding the previous y-transport scheme already had).
"""

import contextlib
import os
import sys

sys.path.insert(0, "/opt/trn_rl_repo")

import ml_dtypes
import numpy as np

import concourse.bass as bass
import concourse.tile as tile
from concourse import bacc, mybir
from concourse.bass_utils import run_bass_kernel_spmd

F32 = mybir.dt.float32
F32R = mybir.dt.float32r
BF16 = mybir.dt.bfloat16

DT_MM_NAME = os.environ.get("TRN_DT_MM", "bf16")
DT_MM = {"f32r": F32R, "bf16": BF16, "f32": F32}[DT_MM_NAME]

B, L, E, H, V = 256, 20, 512, 1024, 10000
T = int(os.environ.get("TRN_T", L + 1))
NCORE = 8
P = 128
HS = H // NCORE          # 128 hidden units per core per layer
VS = V // NCORE          # 1250 vocab slots per core
NKE = E // P             # 4 k-tiles over E
NKH = H // P             # 8 k-tiles over H
BX = B + 2               # gathered row: [h (B) | mean | var]
BN_EPS = 1e-5
# projection N-chunks of VS=1250
NCHUNKS = [(0, 418), (418, 416), (834, 416)]

LAST_EXEC_NS = None
# TRN_FAKE_AG=1 replaces collectives with a local DMA (debug-only)
FAKE_AG = os.environ.get("TRN_FAKE_AG", "0") == "1"

_CACHE = {}


def _fp32r_round(x):
    hi = x.astype(ml_dtypes.bfloat16).astype(np.float32)
    lo = (x - hi).astype(ml_dtypes.bfloat16).astype(np.float32)
    return hi + lo


def build_bass():
    nc = bacc.Bacc("TRN2", target_bir_lowering=False, num_devices=NCORE)
    dt = DT_MM
    ddt = dt

    # ---------------- DRAM I/O ----------------
    # weights/x/masks land in DRAM already in SBUF-tile order (host-side
    # pre-transpose) so every load is a contiguous full-bandwidth DMA
    d_xT = nc.dram_tensor("xT", [T, P, NKE, B], ddt, kind="ExternalInput")
    d_wih1 = nc.dram_tensor("wih1", [P, 4, NKE, HS], ddt, kind="ExternalInput")
    d_whh1 = nc.dram_tensor("whh1", [P, 4, NKH, HS], ddt, kind="ExternalInput")
    d_wih2 = nc.dram_tensor("wih2", [P, 4, NKH, HS], ddt, kind="ExternalInput")
    d_whh2 = nc.dram_tensor("whh2", [P, 4, NKH, HS], ddt, kind="ExternalInput")
    d_woutT = nc.dram_tensor("woutT", [P, NKH, VS], ddt, kind="ExternalInput")
    d_bias1 = nc.dram_tensor("bias1", [HS, 4], F32, kind="ExternalInput")
    d_bias2 = nc.dram_tensor("bias2", [HS, 4], F32, kind="ExternalInput")
    # full-H BN gamma/beta [P, NKH, 2] and full-H masks [NKH, P, B] (bf16):
    # every core derives y for all 1024 units from the gathered h + stats.
    d_gb1 = nc.dram_tensor("gb1", [P, NKH, 2], F32, kind="ExternalInput")
    d_gb2 = nc.dram_tensor("gb2", [P, NKH, 2], F32, kind="ExternalInput")
    d_m1T = nc.dram_tensor("m1T", [P, NKH, B], ddt, kind="ExternalInput")
    d_m2T = nc.dram_tensor("m2T", [P, NKH, B], ddt, kind="ExternalInput")
    d_out = nc.dram_tensor("out", [B * T, VS], F32, kind="ExternalOutput")

    # collective bounce buffers (inputs must be Local, outputs Shared)
    # slot layout: 0 = [h1(t)|stats], 1 = [h2(t-1)|stats]
    RING = 3
    agi = [nc.dram_tensor(f"agi{j}", [2, P, BX], dt, kind="Internal")
           for j in range(RING)]
    ago = [nc.dram_tensor(f"ago{j}", [NCORE, 2, P, BX], dt,
                          kind="Internal", addr_space="Shared")
           for j in range(RING)]

    # SP ring: latency-critical stage + gather readbacks.
    # ACT ring: weights, x loads, projection stores.
    dma_sp = nc.sync.dma_start
    dma_act = nc.scalar.dma_start

    with tile.TileContext(nc) as tc:
        with contextlib.ExitStack() as ctx:
            smalls = ctx.enter_context(tc.tile_pool(name="smalls", bufs=1))
            wts = ctx.enter_context(tc.tile_pool(name="wts", bufs=1))
            xpool = ctx.enter_context(tc.tile_pool(name="xpool", bufs=3))
            gp_h1 = ctx.enter_context(tc.tile_pool(name="g_h1", bufs=2))
            gp_h2 = ctx.enter_context(tc.tile_pool(name="g_h2", bufs=2))
            gp_y1 = ctx.enter_context(tc.tile_pool(name="g_y1", bufs=2))
            gp_y2 = ctx.enter_context(tc.tile_pool(name="g_y2", bufs=2))
            cell = ctx.enter_context(tc.tile_pool(name="cell", bufs=2))
            slpool = ctx.enter_context(tc.tile_pool(name="slp", bufs=3))
            state = ctx.enter_context(tc.tile_pool(name="state", bufs=1))
            psumg = ctx.enter_context(
                tc.tile_pool(name="psumg", bufs=6, space="PSUM"))
            psumP = ctx.enter_context(
                tc.tile_pool(name="psumP", bufs=2, space="PSUM"))
            outp = ctx.enter_context(tc.tile_pool(name="outp", bufs=3))

            # resident weights
            w_ih1 = wts.tile([P, 4, NKE, HS], dt)
            w_hh1 = wts.tile([P, 4, NKH, HS], dt)
            w_ih2 = wts.tile([P, 4, NKH, HS], dt)
            w_hh2 = wts.tile([P, 4, NKH, HS], dt)
            w_out = wts.tile([P, NKH, VS], dt)

            def load_x(t):
                x_t = xpool.tile([P, NKE, B], dt, tag="x", name=f"x_{t}")
                dma_act(x_t[:], d_xT[t][:])
                return x_t

            # prologue: wih1 + x0 unblock the PE first (split across both
            # rings); wih2/whh1/masks follow on ACT.  The two big late-use
            # weights (whh2, w_out) stream in k-chunks from inside the first
            # loop iterations so their transfers ride the collective windows
            # instead of blocking the first stage/readback on the shared
            # DMA fabric.
            dma_sp(w_ih1[:], d_wih1[:])
            x_t = load_x(0)
            dma_act(w_ih2[:], d_wih2[:])
            dma_act(w_hh1[:], d_whh1[:])

            b1 = smalls.tile([HS, 4], F32)
            b2 = smalls.tile([HS, 4], F32)
            gb1 = smalls.tile([P, NKH, 2], F32)
            gb2 = smalls.tile([P, NKH, 2], F32)
            m1 = smalls.tile([P, NKH, B], dt)
            m2 = smalls.tile([P, NKH, B], dt)
            for dst, src in ((b1, d_bias1), (b2, d_bias2),
                             (gb1, d_gb1), (gb2, d_gb2)):
                dma_sp(dst[:], src[:])
            dma_act(m1[:], d_m1T[:])
            dma_act(m2[:], d_m2T[:])

            # persistent state
            c1 = state.tile([P, B], F32)
            c2 = state.tile([P, B], F32)
            nc.vector.memset(c1[:], 0.0)
            nc.vector.memset(c2[:], 0.0)

            def lstm_cell(bias, c_st, pgs, hs_out, s, t):
                """One LSTM cell; writes [h-cast | bn mean,var] into
                hs_out[:, s] (a [P, BX] bf16 staging row).

                psum packing: pgA=(i,g), pgB=(f,o); gate order i=0 f=1 g=2 o=3.
                """
                pgA, pgB = pgs
                i_t = cell.tile([P, B], F32, tag="i", name=f"i_{t}")
                f_t = cell.tile([P, B], F32, tag="f", name=f"f_{t}")
                g_t = cell.tile([P, B], F32, tag="g", name=f"g_{t}")
                o_t = cell.tile([P, B], F32, tag="o", name=f"o_{t}")
                Sig = mybir.ActivationFunctionType.Sigmoid
                Tanh = mybir.ActivationFunctionType.Tanh
                nc.scalar.activation(i_t[:], pgA[:, 0], Sig, bias=bias[:, 0:1])
                nc.scalar.activation(g_t[:], pgA[:, 1], Tanh, bias=bias[:, 2:3])
                nc.scalar.activation(f_t[:], pgB[:, 0], Sig, bias=bias[:, 1:2])
                nc.scalar.activation(o_t[:], pgB[:, 1], Sig, bias=bias[:, 3:4])

                ig = cell.tile([P, B], F32, tag="ig", name=f"ig_{t}")
                nc.vector.tensor_mul(ig[:], i_t[:], g_t[:])
                fc = cell.tile([P, B], F32, tag="fc", name=f"fc_{t}")
                nc.vector.tensor_mul(fc[:], f_t[:], c_st[:])
                nc.vector.tensor_add(c_st[:], ig[:], fc[:])
                tnc = cell.tile([P, B], F32, tag="tc", name=f"tc_{t}")
                nc.scalar.activation(tnc[:], c_st[:], Tanh)
                h_f = cell.tile([P, B], F32, tag="h", name=f"h_{t}")
                nc.vector.tensor_mul(h_f[:], o_t[:], tnc[:])
                nc.scalar.activation(hs_out[:, s, 0:B], h_f[:],
                                     mybir.ActivationFunctionType.Identity)
                # BN stats over batch (free dim); mean/var ride the collective
                st6 = cell.tile([P, 6], F32, tag="st", name=f"st_{t}")
                nc.vector.bn_stats(st6[:], h_f[:])
                mv = cell.tile([P, 2], F32, tag="mv", name=f"mv_{t}")
                nc.vector.bn_aggr(mv[:], st6[:])
                nc.vector.tensor_copy(hs_out[:, s, B:BX], mv[:])

            I32 = mybir.dt.int32

            def bn_derive_span(hgx, gbv, mask, y, klo, khi, t):
                """Derive y[:, klo:khi] = BN(h)*mask from a gathered
                [P, NKH, BX] row-block span.  rsqrt via fast-inverse-sqrt +
                one Newton step, batched over the span's tiles on the DVE;
                per-tile ACT affine (per-partition scale/bias) + Pool-engine
                mask mul.  Split into spans so the first ih2/proj matmuls
                start as soon as the first readback half lands."""
                kn = khi - klo
                mean = hgx[:, klo:khi, B:B + 1]     # [P, kn, 1] bf16 views
                var = hgx[:, klo:khi, B + 1:BX]
                v_t = cell.tile([P, kn, 1], F32, tag="vv", name=f"vv_{t}")
                nc.vector.tensor_scalar_add(v_t[:], var, BN_EPS)
                r_a = cell.tile([P, kn, 1], F32, tag="ra", name=f"ra_{t}")
                ui = cell.tile([P, kn, 1], I32, tag="ui", name=f"ui_{t}")
                nc.vector.tensor_scalar(ui[:], v_t[:].bitcast(I32), 1, None,
                                        op0=mybir.AluOpType.logical_shift_right)
                nc.vector.tensor_scalar(r_a[:].bitcast(I32), ui[:],
                                        -1, 0x5F3759DF,
                                        op0=mybir.AluOpType.mult,
                                        op1=mybir.AluOpType.add)
                rr = cell.tile([P, kn, 1], F32, tag="rr", name=f"rr_{t}")
                ww = cell.tile([P, kn, 1], F32, tag="ww", name=f"ww_{t}")
                r_b = cell.tile([P, kn, 1], F32, tag="rb", name=f"rb_{t}")
                nc.vector.tensor_mul(rr[:], r_a[:], r_a[:])
                nc.vector.scalar_tensor_tensor(
                    ww[:], rr[:], -0.5, v_t[:],
                    op0=mybir.AluOpType.mult, op1=mybir.AluOpType.mult)
                nc.vector.scalar_tensor_tensor(
                    r_b[:], ww[:], 1.5, r_a[:],
                    op0=mybir.AluOpType.add, op1=mybir.AluOpType.mult)
                a_v = cell.tile([P, kn, 1], F32, tag="av", name=f"av_{t}")
                nc.vector.tensor_mul(a_v[:], r_b[:], gbv[:, klo:khi, 0:1])
                ma = cell.tile([P, kn, 1], F32, tag="ma", name=f"ma_{t}")
                nc.vector.tensor_mul(ma[:], mean, a_v[:])
                b_v = cell.tile([P, kn, 1], F32, tag="bv", name=f"bv_{t}")
                nc.vector.tensor_sub(b_v[:], gbv[:, klo:khi, 1:2], ma[:])
                for i, k in enumerate(range(klo, khi)):
                    nc.scalar.activation(
                        y[:, k], hgx[:, k, 0:B],
                        mybir.ActivationFunctionType.Identity,
                        bias=b_v[:, i], scale=a_v[:, i])
                    nc.gpsimd.tensor_mul(y[:, k], y[:, k], mask[:, k])

            def bn_derive(hgx, gbv, mask, ypool, tag, t, halves=False):
                y = ypool.tile([P, NKH, B], dt, tag=tag, name=f"{tag}_{t}")
                if halves:
                    NH = NKH // 2
                    bn_derive_span(hgx, gbv, mask, y, 0, NH, f"{t}a")
                    bn_derive_span(hgx, gbv, mask, y, NH, NKH, f"{t}b")
                else:
                    bn_derive_span(hgx, gbv, mask, y, 0, NKH, t)
                return y

            def emit_ih1(t, x_t, stop):
                # x-side of layer-1 gates for step t.  One accumulation group
                # per PSUM bank: start on the bank's first matmul (gates 0/1),
                # stop on each gate's last matmul (deferred to the hh matmuls
                # unless `stop`).
                pgA = psumg.tile([P, 2, B], F32, tag="pg", name=f"pgA_1_{t}")
                pgB = psumg.tile([P, 2, B], F32, tag="pg", name=f"pgB_1_{t}")
                gloc = {0: (pgA, 0), 2: (pgA, 1), 1: (pgB, 0), 3: (pgB, 1)}
                for gate in (0, 2, 1, 3):
                    tl, sub = gloc[gate]
                    for k in range(NKE):
                        nc.tensor.matmul(
                            tl[:, sub], w_ih1[:, gate, k], x_t[:, k],
                            start=(k == 0 and gate in (0, 1)),
                            stop=(stop and k == NKE - 1 and gate in (2, 3)))
                return pgA, pgB

            def emit_ih2(t, y1g, stop):
                # layer-2 x-side, k-major: consumes y1 tile k for ~0.43us per
                # k, pacing the ACT affines that produce them.
                pgA = psumg.tile([P, 2, B], F32, tag="pg", name=f"pgA_2_{t}")
                pgB = psumg.tile([P, 2, B], F32, tag="pg", name=f"pgB_2_{t}")
                gloc = {0: (pgA, 0), 2: (pgA, 1), 1: (pgB, 0), 3: (pgB, 1)}
                last = NKH - 1
                for k in range(NKH):
                    for gate in (0, 2, 1, 3):
                        tl, sub = gloc[gate]
                        nc.tensor.matmul(
                            tl[:, sub], w_ih2[:, gate, k], y1g[:, k],
                            start=(k == 0 and gate in (0, 1)),
                            stop=(stop and k == last))
                return pgA, pgB

            def emit_hh(pgs, w, hg, widx):
                # recurrent half for either layer, gate-major so each gate's
                # accumulation completes every 8 matmuls and the cell's ACT
                # chain starts early.  hg rows are [P, BX]; h is cols 0:B.
                pgA, pgB = pgs
                gloc = {0: (pgA, 0), 2: (pgA, 1), 1: (pgB, 0), 3: (pgB, 1)}
                for gate in (0, 2, 1, 3):
                    tl, sub = gloc[gate]
                    for k in range(NKH):
                        nc.tensor.matmul(
                            tl[:, sub], w[:, gate, k], hg[:, k, 0:B],
                            start=False, stop=(k == NKH - 1))

            # out rows at step tp as [p, bh, v] (partition-major SBUF view)
            d_out_p = d_out[:].rearrange("(bh p t) v -> t p bh v", t=T, p=P)

            osb_open = {}

            def proj_chunk(tp, y2g_src, piece):
                """One (batch-half, vocab-chunk) piece of step tp's
                projection: 8 matmuls into a double-buffered PSUM bank + one
                drain; the store fires on the last piece.  Drains alternate
                ACT/DVE to keep the ACT ring free for the cell chains."""
                bh, n = piece
                if tp not in osb_open:
                    osb_open[tp] = outp.tile([P, 2, VS], F32, tag="osb",
                                             name=f"osb_{tp}")
                o_sb = osb_open[tp]
                noff, nlen = NCHUNKS[n]
                pp = psumP.tile([P, 512], F32, tag="pp",
                                name=f"pp_{tp}_{bh}_{n}")
                for k in range(NKH):
                    nc.tensor.matmul(
                        pp[:, 0:nlen],
                        y2g_src[:, k, bh * P:(bh + 1) * P],
                        w_out[:, k, noff:noff + nlen],
                        start=(k == 0), stop=(k == NKH - 1))
                if n % 2 == 0:
                    nc.scalar.activation(
                        o_sb[:, bh, noff:noff + nlen], pp[:, 0:nlen],
                        mybir.ActivationFunctionType.Identity)
                else:
                    nc.vector.tensor_copy(
                        o_sb[:, bh, noff:noff + nlen], pp[:, 0:nlen])
                if bh == 1 and n == len(NCHUNKS) - 1:
                    dma_act(d_out_p[tp], o_sb[:])
                    del osb_open[tp]

            PIECES = [(0, 0), (0, 1), (0, 2), (1, 0), (1, 1), (1, 2)]

            def all_gather(idx, dsts):
                """AG on ring slot idx; dsts: (dst_ap, slot) full-row
                readbacks on the SP ring."""
                ai = agi[idx % RING]
                ao = ago[idx % RING]
                if FAKE_AG:
                    dma_sp(ao[0][:], ai[:])
                else:
                    nc.gpsimd.collective_compute(
                        "AllGather", mybir.AluOpType.bypass,
                        ins=[ai[:].opt()], outs=[ao[:].opt()],
                        replica_groups=[list(range(NCORE))])
                agor = ao[:].rearrange("c s p x -> s p c x")
                for dst, slot, klo, khi in dsts:
                    dma_sp(dst, agor[slot, :, klo:khi])

            pending_l1 = emit_ih1(0, x_t, stop=True)
            y2g_prev = None    # y2(t-2), for proj(t-2) in iter t
            y2g_cur = None     # y2(t-1), derived during iter t
            hs_cur = slpool.tile([P, 2, BX], dt, tag="hs", name="hs_0")
            nc.vector.memset(hs_cur[:, 1], 0.0)   # h2(-1) | stats = 0

            for t in range(T):
                # ---- cell1(t): psums completed last iteration ----
                lstm_cell(b1, c1, pending_l1, hs_cur, 0, f"1_{t}")
                agir = agi[t % RING][:].rearrange("s p x -> p s x")
                dma_sp(agir[:], hs_cur[:])

                # ---- the step's single AllGather + readbacks (h1 halves
                # first so the first y1 tiles derive ~1us earlier) ----
                h1gx = gp_h1.tile([P, NKH, BX], dt, tag="h1x", name=f"h1x_{t}")
                NH = NKH // 2
                dsts = [(h1gx[:, 0:NH], 0, 0, NH),
                        (h1gx[:, NH:], 0, NH, NKH)]
                h2gx = None
                if t > 0:
                    h2gx = gp_h2.tile([P, NKH, BX], dt, tag="h2x",
                                      name=f"h2x_{t}")
                    dsts += [(h2gx[:, 0:NH], 1, 0, NH),
                             (h2gx[:, NH:], 1, NH, NKH)]
                all_gather(t, dsts)
                x_nxt = load_x(t + 1) if t + 1 < T else None
                if t == 0:
                    dma_act(w_hh2[:, :, 0:4], d_whh2[:, :, 0:4])
                    dma_act(w_hh2[:, :, 4:8], d_whh2[:, :, 4:8])
                elif t == 1:
                    dma_act(w_out[:, 0:4], d_woutT[:, 0:4])
                    dma_act(w_out[:, 4:8], d_woutT[:, 4:8])

                # ---- PE filler while the AG flies (all AG-independent) ----
                if y2g_prev is not None:
                    proj_chunk(t - 2, y2g_prev, PIECES[0])
                    proj_chunk(t - 2, y2g_prev, PIECES[1])
                if t + 1 < T:
                    pending_l1 = emit_ih1(t + 1, x_nxt, stop=False)
                if y2g_prev is not None:
                    proj_chunk(t - 2, y2g_prev, PIECES[2])
                    proj_chunk(t - 2, y2g_prev, PIECES[3])

                # ---- derive y1(t) from the gathered h1+stats, feed L2 ----
                y1g = bn_derive(h1gx, gb1, m1, gp_y1, "y1g", t,
                                halves=(os.environ.get("TRN_HALVES", "0")
                                        == "1"))
                pending_l2 = emit_ih2(t, y1g, stop=(t == 0))
                # trailing proj pieces right after the ACT-paced ih2: the
                # scheduler weaves them into the y1-production stalls
                if y2g_prev is not None:
                    proj_chunk(t - 2, y2g_prev, PIECES[4])
                    proj_chunk(t - 2, y2g_prev, PIECES[5])
                if t > 0:
                    emit_hh(pending_l2, w_hh2, h2gx, 2)
                if t + 1 < T:
                    emit_hh(pending_l1, w_hh1, h1gx, 1)

                # ---- cell2(t) stages into next step's collective ----
                hs_nxt = slpool.tile([P, 2, BX], dt, tag="hs",
                                     name=f"hs_{t+1}")
                lstm_cell(b2, c2, pending_l2, hs_nxt, 1, f"2_{t}")
                hs_cur = hs_nxt
                if t > 0:
                    y2g_cur = bn_derive(h2gx, gb2, m2, gp_y2, "y2g", t)
                y2g_prev = y2g_cur

            # ---- epilogue: gather h2(T-1); project steps T-2, T-1 ----
            h2gx_last = gp_h2.tile([P, NKH, BX], dt, tag="h2x", name="h2x_T")
            nc.vector.memset(hs_cur[:, 0], 0.0)   # no cell1(T); keep finite
            agir = agi[T % RING][:].rearrange("s p x -> p s x")
            dma_sp(agir[:], hs_cur[:])
            all_gather(T, [(h2gx_last[:], 1, 0, NKH)])
            for piece in PIECES:                     # proj(T-2): y2 derived
                proj_chunk(T - 2, y2g_prev, piece)   # during iter T-1
            y2g_last = bn_derive(h2gx_last, gb2, m2, gp_y2, "y2g", T)
            for piece in PIECES:
                proj_chunk(T - 1, y2g_last, piece)

    nc.finalize()
    return nc


def _prep_inputs(features, captions, lengths, embed_table,
                 W_ih1, W_hh1, b_ih1, b_hh1, gamma1, beta1, mask1,
                 W_ih2, W_hh2, b_ih2, b_hh2, gamma2, beta2, mask2,
                 W_out, b_out):
    f32 = np.float32
    features = np.asarray(features, f32)
    captions = np.asarray(captions)
    embed_table = np.asarray(embed_table, f32)
    if DT_MM == F32R:
        rnd = _fp32r_round
    elif DT_MM == BF16:
        rnd = lambda a: np.ascontiguousarray(a.astype(ml_dtypes.bfloat16))
    else:
        rnd = lambda a: a

    # x sequence [T, B, E] -> xT [T, NKE, P, B]
    x = np.empty((L + 1, B, E), f32)
    x[0] = features
    x[1:] = embed_table[captions].transpose(1, 0, 2)
    x = x[:T]
    # [T, B, E] -> [T, P, NKE, B] (SBUF-tile order, contiguous loads)
    xT = rnd(np.ascontiguousarray(
        x.transpose(0, 2, 1).reshape(T, NKE, P, B).transpose(0, 2, 1, 3)))

    def wslice(Wf, c, K):
        # Wf [4H, K] -> per-core [P, 4, K//P, HS] lhsT blocks
        Wg = np.asarray(Wf, f32).reshape(4, H, K)[:, c * HS:(c + 1) * HS, :]
        w = Wg.transpose(0, 2, 1).reshape(4, K // P, P, HS)
        return rnd(np.ascontiguousarray(w.transpose(2, 0, 1, 3)))

    def gbfull(gamma, beta):
        # [H] pair -> [P, NKH, 2] (tile k, partition p) = unit k*P+p
        g = np.asarray(gamma, f32).reshape(NKH, P).T
        b = np.asarray(beta, f32).reshape(NKH, P).T
        return np.ascontiguousarray(np.stack([g, b], axis=2))

    def maskfull(mask):
        # [B, H] -> [P, NKH, B] bf16
        mT = np.asarray(mask, f32).T.reshape(NKH, P, B).transpose(1, 0, 2)
        return rnd(np.ascontiguousarray(mT))

    bsum1 = (np.asarray(b_ih1, f32) + np.asarray(b_hh1, f32)).reshape(4, H)
    bsum2 = (np.asarray(b_ih2, f32) + np.asarray(b_hh2, f32)).reshape(4, H)
    WoT = np.ascontiguousarray(np.asarray(W_out, f32).T)  # [H, V]

    gb1v, gb2v = gbfull(gamma1, beta1), gbfull(gamma2, beta2)
    m1v, m2v = maskfull(mask1), maskfull(mask2)

    in_maps = []
    for c in range(NCORE):
        u = slice(c * HS, (c + 1) * HS)
        v = slice(c * VS, (c + 1) * VS)
        in_maps.append({
            "xT": xT,
            "wih1": wslice(W_ih1, c, E),
            "whh1": wslice(W_hh1, c, H),
            "wih2": wslice(W_ih2, c, H),
            "whh2": wslice(W_hh2, c, H),
            "woutT": rnd(np.ascontiguousarray(
                WoT[:, v].reshape(NKH, P, VS).transpose(1, 0, 2))),
            "bias1": np.ascontiguousarray(bsum1[:, u].T),
            "bias2": np.ascontiguousarray(bsum2[:, u].T),
            "gb1": gb1v,
            "gb2": gb2v,
            "m1T": m1v,
            "m2T": m2v,
        })
    return in_maps, np.asarray(b_out, f32)


def kernel(**inputs):
    global LAST_EXEC_NS
    if "nc" not in _CACHE:
        _CACHE["nc"] = build_bass()
    nc = _CACHE["nc"]

    in_maps, b_out = _prep_inputs(**inputs)
    trace = os.environ.get("TRN_KERNEL_TRACE", "0") == "1"
    res = run_bass_kernel_spmd(nc, in_maps, core_ids=list(range(NCORE)),
                               trace=trace)
    LAST_EXEC_NS = res.exec_time_ns
    out = np.concatenate([res.results[c]["out"] for c in range(NCORE)], axis=1)
    if b_out.any():
        out = out + b_out[None, :]
    return out
